# revision 21
# baseline (speedup 1.0000x reference)
"""Trainium2 Bass kernel for nn_CompressedInteractionNet_31997506355236.

Reference math (per batch b, channel k, dim d; m == H == 64, D == 16, vk == 16):
    A[bd, kv] = sum_i x0t[i, bd] * Vm[k, i, v]
    B[bd, kv] = sum_j xhrt[j, bd] * Vh[k, v, j]
    out[bd, k] = sum_v A * B

Strategy: 2D sharding, batch x channels = 4 x 2 over 8 cores (32 batches and
32 output channels per core). Inputs ship as fp16 (tolerance is 2e-2; fp16
keeps ~5e-4) in stacked 128-partition layouts, pre-split into contiguous
column halves so every DMA is a fully contiguous 64 KiB transfer:
    xs [128, 512] = [x0t ; xhrt]     ws [128, 512] = [Vm' ; Vh']
Per 128-row chunk c (4 chunks), work is spread over four engines:
    A = xs[0:64,c].T @ ws[0:64]   -> PSUM   (PE, K=64)
    B = xs[64:,c].T @ ws[64:]     -> PSUM   (PE, K=64)
    b_sb = copy(B) -> fp16 SBUF             (ACT, reads PSUM)
    p = A * b_sb -> fp16                    (DVE; one PSUM operand)
    t = p[...,0:8] + p[...,8:16]            (GPSIMD fold, chunks 0-2)
    o = sum_v t (or p on the last chunk)    (DVE grouped reduce)
Chunk 0 is fully half-split so the ACT/DVE chain starts right after the
first 64 KiB of each stream lands; the last chunk reduces v=16 directly on
the DVE for the shortest output tail. A short burst of dummy matmuls on a
zeroed scratch tile keeps the PE busy while the input DMAs are in flight so
the HAM clock gate can open before the real matmuls issue. All DMAs ride
the two HWDGE queues. Output leaves as [(b,d), k_loc]; the host unshards/
transposes.
"""

import numpy as np

import concourse.bass as bass
import concourse.tile as tile
from concourse import bacc, mybir
from concourse.bass_utils import run_bass_kernel_spmd

# Problem constants (hardcoded; kernel must be self-contained).
B, M, D = 128, 64, 16
HK, VK = 64, 16
H = 64
NCORES = 8
SB, SK = 4, 2             # batch shards x channel shards
BL = B // SB              # batches per core = 32
BD = BL * D               # rows per core = 512
KL = HK // SK             # channels per core = 32
KVL = KL * VK             # 512
NCH = BD // 128           # 128-row chunks per core = 4
HKV = KVL // 2
F32 = mybir.dt.float32
F16 = mybir.dt.float16

_CACHE = {}


def build_bass():
    nc = bacc.Bacc("TRN2", target_bir_lowering=False, debug=False,
                   num_devices=NCORES, enable_partition_id=False,
                   monotonic_sem_count=0)

    xsq_d = nc.dram_tensor("xsq", [128, 128], F16, kind="ExternalInput")
    xsr_d = nc.dram_tensor("xsr", [128, 384], F16, kind="ExternalInput")
    wsq_d = nc.dram_tensor("wsq", [128, 128], F16, kind="ExternalInput")
    wsr_d = nc.dram_tensor("wsr", [128, 384], F16, kind="ExternalInput")
    out = nc.dram_tensor("out", [BD, KL], F32, kind="ExternalOutput")

    with tile.TileContext(nc) as tc:
        with (
            tc.tile_pool(name="w", bufs=1) as w,
            tc.tile_pool(name="work", bufs=3) as work,
            tc.tile_pool(name="pab", bufs=3, space="PSUM") as pab,
            tc.tile_pool(name="dpool", bufs=1, space="PSUM") as dpool,
        ):
            ws = w.tile([128, KVL], F16)
            xs = w.tile([128, BD], F16)
            nc.scalar.dma_start(ws[:, 0:128], wsq_d.ap())
            nc.sync.dma_start(xs[:, 0:128], xsq_d.ap())
            nc.scalar.dma_start(ws[:, 128:KVL], wsr_d.ap())
            nc.sync.dma_start(xs[:, 128:BD], xsr_d.ap())

            # PE warm-up: dummy matmuls on a zeroed scratch tile while the
            # loads are in flight, so the HAM clock gate opens earlier.
            dumw = w.tile([128, KVL], F16)
            nc.vector.memset(dumw[:], 0.0)
            dpsum = dpool.tile([128, KVL], F32, tag="dummy")
            for _ in range(3):
                nc.tensor.matmul(dpsum[:], dumw[:, 0:128], dumw[:],
                                 start=True, stop=True)

            for c in range(NCH):
                last = c == NCH - 1
                lhsT_a = xs[0:64, 128 * c:128 * (c + 1)]
                lhsT_b = xs[64:128, 128 * c:128 * (c + 1)]
                psum_a = pab.tile([128, KVL], F32, tag="a")
                psum_b = pab.tile([128, KVL], F32, tag="b")
                b_sb = work.tile([128, KVL], F16, tag="b_sb")
                p = work.tile([128, KL, VK], F16, tag="p")
                o_sb = work.tile([128, KL], F32, tag="o")
                if c == 0:
                    # split pipeline: each kv piece is gated only on its own
                    # ws load piece (32 KiB quarter first), so the ACT/DVE
                    # chain starts as early as possible. The second (big)
                    # piece folds twice on GPSIMD to shrink the DVE reduce.
                    t = work.tile([128, KL, VK // 2], F16, tag="t")
                    t2 = work.tile([128, KL, VK // 4], F16, tag="t2")
                    for ksl, kh, deep in ((slice(0, 128), slice(0, 8), False),
                                          (slice(128, KVL), slice(8, KL),
                                           True)):
                        nc.tensor.matmul(psum_a[:, ksl], lhsT_a, ws[0:64, ksl],
                                         start=True, stop=True)
                        nc.tensor.matmul(psum_b[:, ksl], lhsT_b,
                                         ws[64:128, ksl], start=True, stop=True)
                        nc.scalar.copy(b_sb[:, ksl], psum_b[:, ksl])
                        nc.vector.tensor_mul(
                            out=p[:, kh].rearrange("p k v -> p (k v)"),
                            in0=psum_a[:, ksl], in1=b_sb[:, ksl])
                        nc.gpsimd.tensor_tensor(t[:, kh], p[:, kh, 0:8],
                                                p[:, kh, 8:16],
                                                mybir.AluOpType.add)
                        if deep:
                            nc.gpsimd.tensor_tensor(t2[:, kh], t[:, kh, 0:4],
                                                    t[:, kh, 4:8],
                                                    mybir.AluOpType.add)
                            nc.vector.tensor_reduce(out=o_sb[:, kh],
                                                    in_=t2[:, kh],
                                                    axis=mybir.AxisListType.X,
                                                    op=mybir.AluOpType.add)
                        else:
                            nc.vector.tensor_reduce(out=o_sb[:, kh],
                                                    in_=t[:, kh],
                                                    axis=mybir.AxisListType.X,
                                                    op=mybir.AluOpType.add)
                    nc.sync.dma_start(out.ap()[0:128, :], o_sb[:])
                    continue

                nc.tensor.matmul(psum_a[:], lhsT_a, ws[0:64, :],
                                 start=True, stop=True)
                nc.tensor.matmul(psum_b[:], lhsT_b, ws[64:128, :],
                                 start=True, stop=True)
                nc.scalar.copy(b_sb[:], psum_b[:])
                nc.vector.tensor_mul(out=p.rearrange("p k v -> p (k v)"),
                                     in0=psum_a[:], in1=b_sb[:])
                if last:
                    # tail: direct DVE reduce over v=16 in two k halves so
                    # the first half's store flies while the second reduces
                    for kh in (slice(0, KL // 2), slice(KL // 2, KL)):
                        nc.vector.tensor_reduce(out=o_sb[:, kh], in_=p[:, kh],
                                                axis=mybir.AxisListType.X,
                                                op=mybir.AluOpType.add)
                        nc.sync.dma_start(
                            out.ap()[128 * c:128 * (c + 1), kh], o_sb[:, kh])
                    continue
                # GPSIMD folds v 16->8->4, DVE reduces the rest
                t = work.tile([128, KL, VK // 2], F16, tag="t")
                t2 = work.tile([128, KL, VK // 4], F16, tag="t2")
                nc.gpsimd.tensor_tensor(t[:], p[:, :, 0:8], p[:, :, 8:16],
                                        mybir.AluOpType.add)
                nc.gpsimd.tensor_tensor(t2[:], t[:, :, 0:4], t[:, :, 4:8],
                                        mybir.AluOpType.add)
                nc.vector.tensor_reduce(out=o_sb[:], in_=t2[:],
                                        axis=mybir.AxisListType.X,
                                        op=mybir.AluOpType.add)
                nc.sync.dma_start(out.ap()[128 * c:128 * (c + 1), :], o_sb[:])

    nc.compile()
    return nc


def run(x_0, x_h, Vm, Vh, **spmd_kwargs):
    x_0 = np.ascontiguousarray(np.asarray(x_0), dtype=np.float32)
    vm = np.asarray(Vm)[:, 0].astype(np.float32)   # [k, i, v]
    vh = np.asarray(Vh)[:, 0].astype(np.float32)   # [k, v, j]

    # Host-side layout prep (part of sharding): [i|j, (k,v)] stacked weights.
    vmf = vm.transpose(1, 0, 2).reshape(M, HK * VK)
    vhf = vh.transpose(2, 0, 1).reshape(H, HK * VK)

    if "nc" not in _CACHE:
        _CACHE["nc"] = build_bass()
    nc = _CACHE["nc"]

    in_maps = []
    for core in range(NCORES):
        cb, ck = divmod(core, SK)
        shard = x_0[BL * cb:BL * (cb + 1)]                    # [BL, M, D]
        x0t = shard.transpose(1, 0, 2).reshape(M, BD)         # [i, (b,d)]
        xhrt = shard.reshape(BL, D, H).transpose(2, 0, 1).reshape(H, BD)
        xs = np.concatenate([x0t, xhrt], axis=0).astype(np.float16)
        ks = slice(KVL * ck, KVL * (ck + 1))
        ws = np.concatenate([vmf[:, ks], vhf[:, ks]], axis=0).astype(np.float16)
        in_maps.append({
            "xsq": np.ascontiguousarray(xs[:, 0:128]),
            "xsr": np.ascontiguousarray(xs[:, 128:512]),
            "wsq": np.ascontiguousarray(ws[:, 0:128]),
            "wsr": np.ascontiguousarray(ws[:, 128:512]),
        })

    res = run_bass_kernel_spmd(nc, in_maps, core_ids=list(range(NCORES)),
                               **spmd_kwargs)
    # Unshard: per-core out is [(b,d), k_loc] -> [BL, D, KL] -> [BL, KL, D]
    full = np.empty((B, HK, D), dtype=np.float32)
    for core in range(NCORES):
        cb, ck = divmod(core, SK)
        o = res.results[core]["out"].reshape(BL, D, KL).transpose(0, 2, 1)
        full[BL * cb:BL * (cb + 1), KL * ck:KL * (ck + 1), :] = o
    return full, res


def kernel(x_0, x_h, Vm, Vh):
    return run(x_0, x_h, Vm, Vh)[0]


if __name__ == "__main__":
    rng = np.random.default_rng(0)
    x_0 = rng.standard_normal((B, M, D)).astype(np.float32)
    x_h = rng.standard_normal((B, H, D)).astype(np.float32)
    Vm = rng.standard_normal((HK, 1, M, VK)).astype(np.float32)
    Vh = rng.standard_normal((HK, 1, VK, H)).astype(np.float32)
    got = kernel(x_0, x_h, Vm, Vh)

    x0r = np.transpose(x_0, (0, 2, 1))
    xhr = x_0.reshape(B, D, H)
    a = np.einsum("bdi,kiv->bkdv", x0r, Vm[:, 0])
    bb = np.einsum("bdj,kvj->bkdv", xhr, Vh[:, 0])
    want = np.einsum("bkdv,bkdv->bkd", a, bb)
    err = np.abs(got - want).max() / np.abs(want).max()
    print("rel err:", err)


# revision 23
# speedup vs baseline: 1.1228x; 1.1228x over previous
"""Trainium2 Bass kernel for nn_CompressedInteractionNet_31997506355236.

Reference math (per batch b, channel k, dim d; m == H == 64, D == 16, vk == 16):
    A[bd, kv] = sum_i x0t[i, bd] * Vm[k, i, v]
    B[bd, kv] = sum_j xhrt[j, bd] * Vh[k, v, j]
    out[bd, k] = sum_v A * B

Strategy: 2D sharding, batch x channels = 4 x 2 over 8 cores (32 batches and
32 output channels per core). Inputs ship as fp16 (tolerance is 2e-2; fp16
keeps ~5e-4) in stacked 128-partition layouts, pre-split into contiguous
column halves so every DMA is a fully contiguous 64 KiB transfer:
    xs [128, 512] = [x0t ; xhrt]     ws [128, 512] = [Vm' ; Vh']
Per 128-row chunk c (4 chunks), work is spread over four engines:
    A = xs[0:64,c].T @ ws[0:64]   -> PSUM   (PE, K=64)
    B = xs[64:,c].T @ ws[64:]     -> PSUM   (PE, K=64)
    b_sb = copy(B) -> fp16 SBUF             (ACT, reads PSUM)
    p = A * b_sb -> fp16                    (DVE; one PSUM operand)
    t = p[...,0:8] + p[...,8:16]            (GPSIMD fold, chunks 0-2)
    o = sum_v t (or p on the last chunk)    (DVE grouped reduce)
Chunk 0 is fully half-split so the ACT/DVE chain starts right after the
first 64 KiB of each stream lands; the last chunk reduces v=16 directly on
the DVE for the shortest output tail. A short burst of dummy matmuls on a
zeroed scratch tile keeps the PE busy while the input DMAs are in flight so
the HAM clock gate can open before the real matmuls issue. All DMAs ride
the two HWDGE queues. Output leaves as [(b,d), k_loc]; the host unshards/
transposes.
"""

import numpy as np

import concourse.bass as bass
import concourse.tile as tile
from concourse import bacc, mybir
from concourse.bass_utils import run_bass_kernel_spmd

# Problem constants (hardcoded; kernel must be self-contained).
B, M, D = 128, 64, 16
HK, VK = 64, 16
H = 64
NCORES = 8
SB, SK = 4, 2             # batch shards x channel shards
BL = B // SB              # batches per core = 32
BD = BL * D               # rows per core = 512
KL = HK // SK             # channels per core = 32
KVL = KL * VK             # 512
NCH = BD // 128           # 128-row chunks per core = 4
HKV = KVL // 2
F32 = mybir.dt.float32
F16 = mybir.dt.float16

_CACHE = {}


def build_bass():
    nc = bacc.Bacc("TRN2", target_bir_lowering=False, debug=False,
                   num_devices=NCORES, enable_partition_id=False,
                   monotonic_sem_count=0)

    xsq_d = nc.dram_tensor("xsq", [128, 128], F16, kind="ExternalInput")
    xsr_d = nc.dram_tensor("xsr", [128, 384], F16, kind="ExternalInput")
    wsq_d = nc.dram_tensor("wsq", [128, 128], F16, kind="ExternalInput")
    wsr_d = nc.dram_tensor("wsr", [128, 384], F16, kind="ExternalInput")
    out = nc.dram_tensor("out", [BD, KL], F32, kind="ExternalOutput")

    with tile.TileContext(nc) as tc:
        with (
            tc.tile_pool(name="w", bufs=1) as w,
            tc.tile_pool(name="work", bufs=3) as work,
            tc.tile_pool(name="pab", bufs=3, space="PSUM") as pab,
            tc.tile_pool(name="dpool", bufs=1, space="PSUM") as dpool,
        ):
            ws = w.tile([128, KVL], F16)
            xs = w.tile([128, BD], F16)
            nc.scalar.dma_start(ws[:, 0:128], wsq_d.ap())
            nc.sync.dma_start(xs[:, 0:128], xsq_d.ap())
            nc.scalar.dma_start(ws[:, 128:KVL], wsr_d.ap())
            nc.sync.dma_start(xs[:, 128:BD], xsr_d.ap())

            # PE warm-up: dummy matmuls on a zeroed scratch tile while the
            # loads are in flight, so the HAM clock gate opens earlier.
            dumw = w.tile([128, KVL], F16)
            nc.vector.memset(dumw[:], 0.0)
            dpsum = dpool.tile([128, KVL], F32, tag="dummy")
            for _ in range(3):
                nc.tensor.matmul(dpsum[:], dumw[:, 0:128], dumw[:],
                                 start=True, stop=True)

            for c in range(NCH):
                last = c == NCH - 1
                lhsT_a = xs[0:64, 128 * c:128 * (c + 1)]
                lhsT_b = xs[64:128, 128 * c:128 * (c + 1)]
                psum_a = pab.tile([128, KVL], F32, tag="a")
                psum_b = pab.tile([128, KVL], F32, tag="b")
                b_sb = work.tile([128, KVL], F16, tag="b_sb")
                p = work.tile([128, KL, VK], F16, tag="p")
                o_sb = work.tile([128, KL], F32, tag="o")
                if c == 0:
                    # split pipeline: each kv piece is gated only on its own
                    # ws load piece (32 KiB quarter first), so the ACT/DVE
                    # chain starts as early as possible.
                    t = work.tile([128, KL, VK // 2], F16, tag="t")
                    for ksl, kh in ((slice(0, 128), slice(0, 8)),
                                    (slice(128, KVL), slice(8, KL))):
                        nc.tensor.matmul(psum_a[:, ksl], lhsT_a, ws[0:64, ksl],
                                         start=True, stop=True)
                        nc.tensor.matmul(psum_b[:, ksl], lhsT_b,
                                         ws[64:128, ksl], start=True, stop=True)
                        nc.scalar.copy(b_sb[:, ksl], psum_b[:, ksl])
                        nc.vector.tensor_mul(
                            out=p[:, kh].rearrange("p k v -> p (k v)"),
                            in0=psum_a[:, ksl], in1=b_sb[:, ksl])
                        nc.gpsimd.tensor_tensor(t[:, kh], p[:, kh, 0:8],
                                                p[:, kh, 8:16],
                                                mybir.AluOpType.add)
                        nc.vector.tensor_reduce(out=o_sb[:, kh], in_=t[:, kh],
                                                axis=mybir.AxisListType.X,
                                                op=mybir.AluOpType.add)
                    nc.sync.dma_start(out.ap()[0:128, :], o_sb[:])
                    continue

                nc.tensor.matmul(psum_a[:], lhsT_a, ws[0:64, :],
                                 start=True, stop=True)
                nc.tensor.matmul(psum_b[:], lhsT_b, ws[64:128, :],
                                 start=True, stop=True)
                nc.scalar.copy(b_sb[:], psum_b[:])
                nc.vector.tensor_mul(out=p.rearrange("p k v -> p (k v)"),
                                     in0=psum_a[:], in1=b_sb[:])
                if last:
                    # shortest tail: direct DVE reduce over v=16
                    nc.vector.tensor_reduce(out=o_sb[:], in_=p[:],
                                            axis=mybir.AxisListType.X,
                                            op=mybir.AluOpType.add)
                else:
                    # GPSIMD folds v 16->8, DVE reduces the rest
                    t = work.tile([128, KL, VK // 2], F16, tag="t")
                    nc.gpsimd.tensor_tensor(t[:], p[:, :, 0:8], p[:, :, 8:16],
                                            mybir.AluOpType.add)
                    nc.vector.tensor_reduce(out=o_sb[:], in_=t[:],
                                            axis=mybir.AxisListType.X,
                                            op=mybir.AluOpType.add)
                nc.sync.dma_start(out.ap()[128 * c:128 * (c + 1), :], o_sb[:])

    nc.compile()
    return nc


def run(x_0, x_h, Vm, Vh, **spmd_kwargs):
    x_0 = np.ascontiguousarray(np.asarray(x_0), dtype=np.float32)
    vm = np.asarray(Vm)[:, 0].astype(np.float32)   # [k, i, v]
    vh = np.asarray(Vh)[:, 0].astype(np.float32)   # [k, v, j]

    # Host-side layout prep (part of sharding): [i|j, (k,v)] stacked weights.
    vmf = vm.transpose(1, 0, 2).reshape(M, HK * VK)
    vhf = vh.transpose(2, 0, 1).reshape(H, HK * VK)

    if "nc" not in _CACHE:
        _CACHE["nc"] = build_bass()
    nc = _CACHE["nc"]

    in_maps = []
    for core in range(NCORES):
        cb, ck = divmod(core, SK)
        shard = x_0[BL * cb:BL * (cb + 1)]                    # [BL, M, D]
        x0t = shard.transpose(1, 0, 2).reshape(M, BD)         # [i, (b,d)]
        xhrt = shard.reshape(BL, D, H).transpose(2, 0, 1).reshape(H, BD)
        xs = np.concatenate([x0t, xhrt], axis=0).astype(np.float16)
        ks = slice(KVL * ck, KVL * (ck + 1))
        ws = np.concatenate([vmf[:, ks], vhf[:, ks]], axis=0).astype(np.float16)
        in_maps.append({
            "xsq": np.ascontiguousarray(xs[:, 0:128]),
            "xsr": np.ascontiguousarray(xs[:, 128:512]),
            "wsq": np.ascontiguousarray(ws[:, 0:128]),
            "wsr": np.ascontiguousarray(ws[:, 128:512]),
        })

    res = run_bass_kernel_spmd(nc, in_maps, core_ids=list(range(NCORES)),
                               **spmd_kwargs)
    # Unshard: per-core out is [(b,d), k_loc] -> [BL, D, KL] -> [BL, KL, D]
    full = np.empty((B, HK, D), dtype=np.float32)
    for core in range(NCORES):
        cb, ck = divmod(core, SK)
        o = res.results[core]["out"].reshape(BL, D, KL).transpose(0, 2, 1)
        full[BL * cb:BL * (cb + 1), KL * ck:KL * (ck + 1), :] = o
    return full, res


def kernel(x_0, x_h, Vm, Vh):
    return run(x_0, x_h, Vm, Vh)[0]


if __name__ == "__main__":
    rng = np.random.default_rng(0)
    x_0 = rng.standard_normal((B, M, D)).astype(np.float32)
    x_h = rng.standard_normal((B, H, D)).astype(np.float32)
    Vm = rng.standard_normal((HK, 1, M, VK)).astype(np.float32)
    Vh = rng.standard_normal((HK, 1, VK, H)).astype(np.float32)
    got = kernel(x_0, x_h, Vm, Vh)

    x0r = np.transpose(x_0, (0, 2, 1))
    xhr = x_0.reshape(B, D, H)
    a = np.einsum("bdi,kiv->bkdv", x0r, Vm[:, 0])
    bb = np.einsum("bdj,kvj->bkdv", xhr, Vh[:, 0])
    want = np.einsum("bkdv,bkdv->bkd", a, bb)
    err = np.abs(got - want).max() / np.abs(want).max()
    print("rel err:", err)


# revision 25
# speedup vs baseline: 1.2492x; 1.1126x over previous
"""Trainium2 Bass kernel for nn_CompressedInteractionNet_31997506355236.

Reference math (per batch b, channel k, dim d; m == H == 64, D == 16, vk == 16):
    A[bd, kv] = sum_i x0t[i, bd] * Vm[k, i, v]
    B[bd, kv] = sum_j xhrt[j, bd] * Vh[k, v, j]
    out[bd, k] = sum_v A * B

Strategy: 2D sharding, batch x channels = 4 x 2 over 8 cores (32 batches and
32 output channels per core). Inputs ship as fp16 (tolerance is 2e-2; fp16
keeps ~5e-4) in stacked 128-partition layouts, pre-split into contiguous
column halves so every DMA is a fully contiguous 64 KiB transfer:
    xs [128, 512] = [x0t ; xhrt]     ws [128, 512] = [Vm' ; Vh']
Per 128-row chunk c (4 chunks), work is spread over four engines:
    A = xs[0:64,c].T @ ws[0:64]   -> PSUM   (PE, K=64)
    B = xs[64:,c].T @ ws[64:]     -> PSUM   (PE, K=64)
    b_sb = copy(B) -> fp16 SBUF             (ACT, reads PSUM)
    p = A * b_sb -> fp16                    (DVE; one PSUM operand)
    t = p[...,0:8] + p[...,8:16]            (GPSIMD fold, chunks 0-2)
    o = sum_v t (or p on the last chunk)    (DVE grouped reduce)
Chunk 0 is fully half-split so the ACT/DVE chain starts right after the
first 64 KiB of each stream lands; the last chunk reduces v=16 directly on
the DVE for the shortest output tail. A short burst of dummy matmuls on a
zeroed scratch tile keeps the PE busy while the input DMAs are in flight so
the HAM clock gate can open before the real matmuls issue. All DMAs ride
the two HWDGE queues. Output leaves as [(b,d), k_loc]; the host unshards/
transposes.
"""

import numpy as np

import concourse.bass as bass
import concourse.tile as tile
from concourse import bacc, mybir
from concourse.bass_utils import run_bass_kernel_spmd

# Problem constants (hardcoded; kernel must be self-contained).
B, M, D = 128, 64, 16
HK, VK = 64, 16
H = 64
NCORES = 8
SB, SK = 4, 2             # batch shards x channel shards
BL = B // SB              # batches per core = 32
BD = BL * D               # rows per core = 512
KL = HK // SK             # channels per core = 32
KVL = KL * VK             # 512
NCH = BD // 128           # 128-row chunks per core = 4
HKV = KVL // 2
F32 = mybir.dt.float32
F16 = mybir.dt.float16

_CACHE = {}


def build_bass():
    nc = bacc.Bacc("TRN2", target_bir_lowering=False, debug=False,
                   num_devices=NCORES, enable_partition_id=False,
                   monotonic_sem_count=0)

    xsq_d = nc.dram_tensor("xsq", [128, 128], F16, kind="ExternalInput")
    xsr_d = nc.dram_tensor("xsr", [128, 384], F16, kind="ExternalInput")
    wsq_d = nc.dram_tensor("wsq", [128, 128], F16, kind="ExternalInput")
    wsr_d = nc.dram_tensor("wsr", [128, 384], F16, kind="ExternalInput")
    out = nc.dram_tensor("out", [BD, KL], F32, kind="ExternalOutput")

    with tile.TileContext(nc) as tc:
        with (
            tc.tile_pool(name="w", bufs=1) as w,
            tc.tile_pool(name="work", bufs=3) as work,
            tc.tile_pool(name="pab", bufs=4, space="PSUM") as pab,
        ):
            ws = w.tile([128, KVL], F16)
            xs = w.tile([128, BD], F16)
            nc.scalar.dma_start(ws[:, 0:128], wsq_d.ap())
            nc.sync.dma_start(xs[:, 0:128], xsq_d.ap())
            nc.scalar.dma_start(ws[:, 128:KVL], wsr_d.ap())
            nc.sync.dma_start(xs[:, 128:BD], xsr_d.ap())

            # PE warm-up: dummy matmuls on a zeroed scratch tile while the
            # loads are in flight, so the HAM clock gate opens earlier.
            # The dummy target takes one slot of the "a" PSUM ring (4 bufs x
            # 2 tags = all 8 banks); chunk 3 recycles this slot, which is
            # free long before its matmuls are ready.
            dumw = w.tile([128, KVL], F16)
            nc.vector.memset(dumw[:], 0.0)
            dpsum = pab.tile([128, KVL], F32, tag="a")
            for _ in range(3):
                nc.tensor.matmul(dpsum[:], dumw[:, 0:128], dumw[:],
                                 start=True, stop=True)

            for c in range(NCH):
                last = c == NCH - 1
                lhsT_a = xs[0:64, 128 * c:128 * (c + 1)]
                lhsT_b = xs[64:128, 128 * c:128 * (c + 1)]
                psum_a = pab.tile([128, KVL], F32, tag="a")
                psum_b = pab.tile([128, KVL], F32, tag="b")
                b_sb = work.tile([128, KVL], F16, tag="b_sb")
                p = work.tile([128, KL, VK], F16, tag="p")
                o_sb = work.tile([128, KL], F32, tag="o")
                if c == 0:
                    # split pipeline: each kv piece is gated only on its own
                    # ws load piece (32 KiB quarter first), so the ACT/DVE
                    # chain starts as early as possible.
                    t = work.tile([128, KL, VK // 2], F16, tag="t")
                    for ksl, kh in ((slice(0, 128), slice(0, 8)),
                                    (slice(128, KVL), slice(8, KL))):
                        nc.tensor.matmul(psum_a[:, ksl], lhsT_a, ws[0:64, ksl],
                                         start=True, stop=True)
                        nc.tensor.matmul(psum_b[:, ksl], lhsT_b,
                                         ws[64:128, ksl], start=True, stop=True)
                        nc.scalar.copy(b_sb[:, ksl], psum_b[:, ksl])
                        nc.vector.tensor_mul(
                            out=p[:, kh].rearrange("p k v -> p (k v)"),
                            in0=psum_a[:, ksl], in1=b_sb[:, ksl])
                        nc.gpsimd.tensor_tensor(t[:, kh], p[:, kh, 0:8],
                                                p[:, kh, 8:16],
                                                mybir.AluOpType.add)
                        nc.vector.tensor_reduce(out=o_sb[:, kh], in_=t[:, kh],
                                                axis=mybir.AxisListType.X,
                                                op=mybir.AluOpType.add)
                    nc.sync.dma_start(out.ap()[0:128, :], o_sb[:])
                    continue

                nc.tensor.matmul(psum_a[:], lhsT_a, ws[0:64, :],
                                 start=True, stop=True)
                nc.tensor.matmul(psum_b[:], lhsT_b, ws[64:128, :],
                                 start=True, stop=True)
                nc.scalar.copy(b_sb[:], psum_b[:])
                nc.vector.tensor_mul(out=p.rearrange("p k v -> p (k v)"),
                                     in0=psum_a[:], in1=b_sb[:])
                if last:
                    # shortest tail: direct DVE reduce over v=16
                    nc.vector.tensor_reduce(out=o_sb[:], in_=p[:],
                                            axis=mybir.AxisListType.X,
                                            op=mybir.AluOpType.add)
                else:
                    # GPSIMD folds v 16->8, DVE reduces the rest
                    t = work.tile([128, KL, VK // 2], F16, tag="t")
                    nc.gpsimd.tensor_tensor(t[:], p[:, :, 0:8], p[:, :, 8:16],
                                            mybir.AluOpType.add)
                    nc.vector.tensor_reduce(out=o_sb[:], in_=t[:],
                                            axis=mybir.AxisListType.X,
                                            op=mybir.AluOpType.add)
                nc.sync.dma_start(out.ap()[128 * c:128 * (c + 1), :], o_sb[:])

    nc.compile()
    return nc


def run(x_0, x_h, Vm, Vh, **spmd_kwargs):
    x_0 = np.ascontiguousarray(np.asarray(x_0), dtype=np.float32)
    vm = np.asarray(Vm)[:, 0].astype(np.float32)   # [k, i, v]
    vh = np.asarray(Vh)[:, 0].astype(np.float32)   # [k, v, j]

    # Host-side layout prep (part of sharding): [i|j, (k,v)] stacked weights.
    vmf = vm.transpose(1, 0, 2).reshape(M, HK * VK)
    vhf = vh.transpose(2, 0, 1).reshape(H, HK * VK)

    if "nc" not in _CACHE:
        _CACHE["nc"] = build_bass()
    nc = _CACHE["nc"]

    in_maps = []
    for core in range(NCORES):
        cb, ck = divmod(core, SK)
        shard = x_0[BL * cb:BL * (cb + 1)]                    # [BL, M, D]
        x0t = shard.transpose(1, 0, 2).reshape(M, BD)         # [i, (b,d)]
        xhrt = shard.reshape(BL, D, H).transpose(2, 0, 1).reshape(H, BD)
        xs = np.concatenate([x0t, xhrt], axis=0).astype(np.float16)
        ks = slice(KVL * ck, KVL * (ck + 1))
        ws = np.concatenate([vmf[:, ks], vhf[:, ks]], axis=0).astype(np.float16)
        in_maps.append({
            "xsq": np.ascontiguousarray(xs[:, 0:128]),
            "xsr": np.ascontiguousarray(xs[:, 128:512]),
            "wsq": np.ascontiguousarray(ws[:, 0:128]),
            "wsr": np.ascontiguousarray(ws[:, 128:512]),
        })

    res = run_bass_kernel_spmd(nc, in_maps, core_ids=list(range(NCORES)),
                               **spmd_kwargs)
    # Unshard: per-core out is [(b,d), k_loc] -> [BL, D, KL] -> [BL, KL, D]
    full = np.empty((B, HK, D), dtype=np.float32)
    for core in range(NCORES):
        cb, ck = divmod(core, SK)
        o = res.results[core]["out"].reshape(BL, D, KL).transpose(0, 2, 1)
        full[BL * cb:BL * (cb + 1), KL * ck:KL * (ck + 1), :] = o
    return full, res


def kernel(x_0, x_h, Vm, Vh):
    return run(x_0, x_h, Vm, Vh)[0]


if __name__ == "__main__":
    rng = np.random.default_rng(0)
    x_0 = rng.standard_normal((B, M, D)).astype(np.float32)
    x_h = rng.standard_normal((B, H, D)).astype(np.float32)
    Vm = rng.standard_normal((HK, 1, M, VK)).astype(np.float32)
    Vh = rng.standard_normal((HK, 1, VK, H)).astype(np.float32)
    got = kernel(x_0, x_h, Vm, Vh)

    x0r = np.transpose(x_0, (0, 2, 1))
    xhr = x_0.reshape(B, D, H)
    a = np.einsum("bdi,kiv->bkdv", x0r, Vm[:, 0])
    bb = np.einsum("bdj,kvj->bkdv", xhr, Vh[:, 0])
    want = np.einsum("bkdv,bkdv->bkd", a, bb)
    err = np.abs(got - want).max() / np.abs(want).max()
    print("rel err:", err)


# revision 27
# speedup vs baseline: 1.2510x; 1.0014x over previous
"""Trainium2 Bass kernel for nn_CompressedInteractionNet_31997506355236.

Reference math (per batch b, channel k, dim d; m == H == 64, D == 16, vk == 16):
    A[bd, kv] = sum_i x0t[i, bd] * Vm[k, i, v]
    B[bd, kv] = sum_j xhrt[j, bd] * Vh[k, v, j]
    out[bd, k] = sum_v A * B

Strategy: 2D sharding, batch x channels = 4 x 2 over 8 cores (32 batches and
32 output channels per core). Inputs ship as fp16 (tolerance is 2e-2; fp16
keeps ~5e-4) in stacked 128-partition layouts, pre-split into contiguous
column halves so every DMA is a fully contiguous 64 KiB transfer:
    xs [128, 512] = [x0t ; xhrt]     ws [128, 512] = [Vm' ; Vh']
Per 128-row chunk c (4 chunks), work is spread over four engines:
    A = xs[0:64,c].T @ ws[0:64]   -> PSUM   (PE, K=64)
    B = xs[64:,c].T @ ws[64:]     -> PSUM   (PE, K=64)
    b_sb = copy(B) -> fp16 SBUF             (ACT, reads PSUM)
    p = A * b_sb -> fp16                    (DVE; one PSUM operand)
    t = p[...,0:8] + p[...,8:16]            (GPSIMD fold, chunks 0-2)
    o = sum_v t (or p on the last chunk)    (DVE grouped reduce)
Chunk 0 is fully half-split so the ACT/DVE chain starts right after the
first 64 KiB of each stream lands; the last chunk reduces v=16 directly on
the DVE for the shortest output tail. A short burst of dummy matmuls on a
zeroed scratch tile keeps the PE busy while the input DMAs are in flight so
the HAM clock gate can open before the real matmuls issue. All DMAs ride
the two HWDGE queues. Output leaves as [(b,d), k_loc]; the host unshards/
transposes.
"""

import numpy as np

import concourse.bass as bass
import concourse.tile as tile
from concourse import bacc, mybir
from concourse.bass_utils import run_bass_kernel_spmd

# Problem constants (hardcoded; kernel must be self-contained).
B, M, D = 128, 64, 16
HK, VK = 64, 16
H = 64
NCORES = 8
SB, SK = 4, 2             # batch shards x channel shards
BL = B // SB              # batches per core = 32
BD = BL * D               # rows per core = 512
KL = HK // SK             # channels per core = 32
KVL = KL * VK             # 512
NCH = BD // 128           # 128-row chunks per core = 4
HKV = KVL // 2
F32 = mybir.dt.float32
F16 = mybir.dt.float16

_CACHE = {}


def build_bass():
    nc = bacc.Bacc("TRN2", target_bir_lowering=False, debug=False,
                   num_devices=NCORES, enable_partition_id=False,
                   monotonic_sem_count=0)

    xsq_d = nc.dram_tensor("xsq", [128, 128], F16, kind="ExternalInput")
    xsr_d = nc.dram_tensor("xsr", [128, 384], F16, kind="ExternalInput")
    wsq_d = nc.dram_tensor("wsq", [128, 128], F16, kind="ExternalInput")
    wsr_d = nc.dram_tensor("wsr", [128, 384], F16, kind="ExternalInput")
    out = nc.dram_tensor("out", [BD, KL], F32, kind="ExternalOutput")

    with tile.TileContext(nc) as tc:
        with (
            tc.tile_pool(name="w", bufs=1) as w,
            tc.tile_pool(name="work", bufs=3) as work,
            tc.tile_pool(name="pab", bufs=3, space="PSUM") as pab,
            tc.tile_pool(name="dpool", bufs=1, space="PSUM") as dpool,
        ):
            ws = w.tile([128, KVL], F16)
            xs = w.tile([128, BD], F16)
            nc.scalar.dma_start(ws[:, 0:128], wsq_d.ap())
            nc.sync.dma_start(xs[:, 0:128], xsq_d.ap())
            nc.scalar.dma_start(ws[:, 128:KVL], wsr_d.ap())
            nc.sync.dma_start(xs[:, 128:BD], xsr_d.ap())

            # PE warm-up: dummy matmuls on a zeroed scratch tile while the
            # loads are in flight, so the HAM clock gate opens earlier.
            dumw = w.tile([128, KVL], F16)
            nc.vector.memset(dumw[:], 0.0)
            dpsum = dpool.tile([128, KVL], F32, tag="dummy")
            for _ in range(3):
                nc.tensor.matmul(dpsum[:], dumw[:, 0:128], dumw[:],
                                 start=True, stop=True)

            for c in range(NCH):
                last = c == NCH - 1
                lhsT_a = xs[0:64, 128 * c:128 * (c + 1)]
                lhsT_b = xs[64:128, 128 * c:128 * (c + 1)]
                psum_a = pab.tile([128, KVL], F32, tag="a")
                psum_b = pab.tile([128, KVL], F32, tag="b")
                b_sb = work.tile([128, KVL], F16, tag="b_sb")
                p = work.tile([128, KL, VK], F16, tag="p")
                o_sb = work.tile([128, KL], F32, tag="o")
                if c == 0:
                    # split pipeline: each kv piece is gated only on its own
                    # ws load piece (32 KiB quarter first), so the ACT/DVE
                    # chain starts as early as possible.
                    t = work.tile([128, KL, VK // 2], F16, tag="t")
                    for ksl, kh in ((slice(0, 128), slice(0, 8)),
                                    (slice(128, KVL), slice(8, KL))):
                        nc.tensor.matmul(psum_a[:, ksl], lhsT_a, ws[0:64, ksl],
                                         start=True, stop=True)
                        nc.tensor.matmul(psum_b[:, ksl], lhsT_b,
                                         ws[64:128, ksl], start=True, stop=True)
                        nc.scalar.copy(b_sb[:, ksl], psum_b[:, ksl])
                        nc.vector.tensor_mul(
                            out=p[:, kh].rearrange("p k v -> p (k v)"),
                            in0=psum_a[:, ksl], in1=b_sb[:, ksl])
                        nc.gpsimd.tensor_tensor(t[:, kh], p[:, kh, 0:8],
                                                p[:, kh, 8:16],
                                                mybir.AluOpType.add)
                        nc.vector.tensor_reduce(out=o_sb[:, kh], in_=t[:, kh],
                                                axis=mybir.AxisListType.X,
                                                op=mybir.AluOpType.add)
                    nc.sync.dma_start(out.ap()[0:128, :], o_sb[:])
                    continue

                nc.tensor.matmul(psum_a[:], lhsT_a, ws[0:64, :],
                                 start=True, stop=True)
                nc.tensor.matmul(psum_b[:], lhsT_b, ws[64:128, :],
                                 start=True, stop=True)
                nc.scalar.copy(b_sb[:], psum_b[:])
                nc.vector.tensor_mul(out=p.rearrange("p k v -> p (k v)"),
                                     in0=psum_a[:], in1=b_sb[:])
                if last:
                    # shortest tail: direct DVE reduce over v=16
                    nc.vector.tensor_reduce(out=o_sb[:], in_=p[:],
                                            axis=mybir.AxisListType.X,
                                            op=mybir.AluOpType.add)
                else:
                    # GPSIMD folds v 16->8, DVE reduces the rest
                    t = work.tile([128, KL, VK // 2], F16, tag="t")
                    nc.gpsimd.tensor_tensor(t[:], p[:, :, 0:8], p[:, :, 8:16],
                                            mybir.AluOpType.add)
                    nc.vector.tensor_reduce(out=o_sb[:], in_=t[:],
                                            axis=mybir.AxisListType.X,
                                            op=mybir.AluOpType.add)
                nc.sync.dma_start(out.ap()[128 * c:128 * (c + 1), :], o_sb[:])

    nc.compile()
    return nc


def run(x_0, x_h, Vm, Vh, **spmd_kwargs):
    x_0 = np.ascontiguousarray(np.asarray(x_0), dtype=np.float32)
    vm = np.asarray(Vm)[:, 0].astype(np.float32)   # [k, i, v]
    vh = np.asarray(Vh)[:, 0].astype(np.float32)   # [k, v, j]

    # Host-side layout prep (part of sharding): [i|j, (k,v)] stacked weights.
    vmf = vm.transpose(1, 0, 2).reshape(M, HK * VK)
    vhf = vh.transpose(2, 0, 1).reshape(H, HK * VK)

    if "nc" not in _CACHE:
        _CACHE["nc"] = build_bass()
    nc = _CACHE["nc"]

    in_maps = []
    for core in range(NCORES):
        cb, ck = divmod(core, SK)
        shard = x_0[BL * cb:BL * (cb + 1)]                    # [BL, M, D]
        x0t = shard.transpose(1, 0, 2).reshape(M, BD)         # [i, (b,d)]
        xhrt = shard.reshape(BL, D, H).transpose(2, 0, 1).reshape(H, BD)
        xs = np.concatenate([x0t, xhrt], axis=0).astype(np.float16)
        ks = slice(KVL * ck, KVL * (ck + 1))
        ws = np.concatenate([vmf[:, ks], vhf[:, ks]], axis=0).astype(np.float16)
        in_maps.append({
            "xsq": np.ascontiguousarray(xs[:, 0:128]),
            "xsr": np.ascontiguousarray(xs[:, 128:512]),
            "wsq": np.ascontiguousarray(ws[:, 0:128]),
            "wsr": np.ascontiguousarray(ws[:, 128:512]),
        })

    res = run_bass_kernel_spmd(nc, in_maps, core_ids=list(range(NCORES)),
                               **spmd_kwargs)
    # Unshard: per-core out is [(b,d), k_loc] -> [BL, D, KL] -> [BL, KL, D]
    full = np.empty((B, HK, D), dtype=np.float32)
    for core in range(NCORES):
        cb, ck = divmod(core, SK)
        o = res.results[core]["out"].reshape(BL, D, KL).transpose(0, 2, 1)
        full[BL * cb:BL * (cb + 1), KL * ck:KL * (ck + 1), :] = o
    return full, res


def kernel(x_0, x_h, Vm, Vh):
    return run(x_0, x_h, Vm, Vh)[0]


if __name__ == "__main__":
    rng = np.random.default_rng(0)
    x_0 = rng.standard_normal((B, M, D)).astype(np.float32)
    x_h = rng.standard_normal((B, H, D)).astype(np.float32)
    Vm = rng.standard_normal((HK, 1, M, VK)).astype(np.float32)
    Vh = rng.standard_normal((HK, 1, VK, H)).astype(np.float32)
    got = kernel(x_0, x_h, Vm, Vh)

    x0r = np.transpose(x_0, (0, 2, 1))
    xhr = x_0.reshape(B, D, H)
    a = np.einsum("bdi,kiv->bkdv", x0r, Vm[:, 0])
    bb = np.einsum("bdj,kvj->bkdv", xhr, Vh[:, 0])
    want = np.einsum("bkdv,bkdv->bkd", a, bb)
    err = np.abs(got - want).max() / np.abs(want).max()
    print("rel err:", err)
